# revision 7
# baseline (speedup 1.0000x reference)
"""HRR attention kernel for 8 Trainium2 NeuronCores (axon-tunneled).

The axon host<->device tunnel is the bottleneck (~40 MB/s each way,
serialized across devices, ~110 ms fixed dispatch overhead per SPMD
launch; the host has a single CPU core), so the kernel minimizes wire
bytes and overlaps host work with wire time:

  H2D: ONE uint8 payload [8, PAY] (~15.7 MB), row-sharded, carrying
    - q/k/v int4 (per-64-block scales), two nibbles per byte
    - Wq/Wk/Wv/Wo int4 (per-64-block scales), 128 rows per core
    - scales and biases as uint16 fixed-point lo/hi uint8 planes
  Packing runs per-core on the CPU and each core's shard is put
  asynchronously as soon as it is ready, so pack time hides under the
  serialized wire time of earlier shards.
  D2H: ONE uint8 array [8192, 544]: int4 nibbles of attn @ Wo.T
  WITHOUT bo (the output is ~99% bo; bo is added host-side in f32, so
  the quantization scale only spans the small attention part) plus
  uint16 per-64-block scale planes. Shards are fetched sequentially
  while already-fetched shards unpack on the CPU.

Quantization error budget (vs the CPU reference): int4 qkv+W ~1.9e-3,
int4 output sans bo ~1.1e-3, device compute ~2e-4; total ~2e-3 against
the 2e-2 gate.

Sharding: rows of the flattened [B*S=8192, D] tensors, 1024 rows/core;
core 2b holds batch b s<1024, core 2b+1 batch b s>=1024. Cross-core
reductions (bind-stage sum over S, softmax over S) are psums over core
pairs [[0,1],[2,3],[4,5],[6,7]]. Weight shards all-gather on fabric.

FFT bind/unbind are reformulated as tiny matmuls with one-hot circulant
tensors built on-device from iotas:
  circconv(x, y)[j] = sum_i x[i] y[(j-i)%64]
  bind:   beta[h,j] = sum_{i,m:(i+m)%64==j} G[h,i,m],  G = kp^T @ vp
  unbind: v_hat = qt @ C(beta), C(beta)[m,j] = beta[(j-m)%64]
  approx_transpose: qt = qp @ P, P[i,j] = 1 iff (i+j)%64 == 0.
"""

import time
import numpy as np
import jax
import jax.numpy as jnp
from jax.sharding import Mesh, NamedSharding, PartitionSpec as P
from functools import partial
from concurrent.futures import ThreadPoolExecutor

try:
    from jax import shard_map
    _SM_KW = {'check_vma': False}
except ImportError:
    from jax.experimental.shard_map import shard_map
    _SM_KW = {'check_rep': False}

try:
    jax.config.update("jax_compilation_cache_dir", "/tmp/jax_comp_cache")
    jax.config.update("jax_persistent_cache_min_compile_time_secs", 10.0)
except Exception:
    pass

B, S, D = 4, 2048, 1024
H, Hd = 16, 64
EPS = 1e-8
N = 8
ROWS = B * S // N              # 1024 rows per core
WROWS = D // N                 # 128 weight rows per core
PAIRS = [[0, 1], [2, 3], [4, 5], [6, 7]]

# fixed-point quanta for uint16-encoded scales/biases (clamped on encode)
SQ_QKV = 1e-5                  # qkv block scales ~0.38, max 0.655
SQ_W = 1e-6                    # W int4 block scales ~0.0076, max 0.0655
SQ_B = 4e-6                    # biases ~N(0,0.02^2), offset-binary
SQ_OUT = 1e-6                  # output block scales << 0.0655

# per-core payload layout (offsets in bytes)
_QNIB = ROWS * (D // 2)        # 524288 per qkv tensor
_SCL = ROWS * 32               # 32768: scale lo/hi planes
_WNIB = WROWS * (D // 2)       # 65536 per weight (int4)
_WSCL = WROWS * 32             # 4096 per weight
_BPL = 4 * 2 * D               # 8192: 4 biases, lo+hi planes
OFF_Q, OFF_K, OFF_V = 0, _QNIB, 2 * _QNIB
OFF_QS = 3 * _QNIB
OFF_KS = OFF_QS + _SCL
OFF_VS = OFF_KS + _SCL
OFF_W = OFF_VS + _SCL          # 4 weights contiguous
OFF_WS = OFF_W + 4 * _WNIB
OFF_B = OFF_WS + 4 * _WSCL
PAY = OFF_B + _BPL             # 1957888 (~1.87 MB/core)

OUT_COLS = D // 2 + 32         # 544


_mesh = None
_sh_pay = None
_cpu = None


def _init_mesh():
    global _mesh, _sh_pay
    if _mesh is None:
        devs = jax.devices()[:N]
        _mesh = Mesh(np.array(devs), ('x',))
        _sh_pay = NamedSharding(_mesh, P('x', None))
    return _mesh, _sh_pay


def _get_cpu():
    global _cpu
    if _cpu is None:
        _cpu = jax.devices('cpu')[0]
    return _cpu


# ---------------- host-side pack (per-core, jit on CPU) ----------------

def _nib_pack(n):
    """n [R,1024] uint8 in [1,15] -> [R,512] packed (byte j = n[j] | n[512+j]<<4)."""
    return n[:, :D // 2] | (n[:, D // 2:] << 4)


def _enc_u16(v, quant):
    """v [R,16] f32 -> [R,32] uint8 lo|hi planes of round(v/quant)."""
    e = jnp.clip(jnp.round(v / quant), 0, 65535).astype(jnp.uint32)
    return jnp.concatenate([(e & 255).astype(jnp.uint8),
                            (e >> 8).astype(jnp.uint8)], axis=1)


def _quant4(x, quant):
    """x [R,1024] f32 -> packed nibbles [R,512], scale planes [R,32]."""
    xb = x.reshape(-1, H, Hd)
    am = jnp.max(jnp.abs(xb), axis=2)
    s = jnp.maximum(am / 7.0, quant)
    n = jnp.clip(jnp.round(xb / s[:, :, None]), -7, 7) + 8
    return _nib_pack(n.reshape(-1, D).astype(jnp.uint8)), _enc_u16(s, quant)


@partial(jax.jit, backend='cpu')
def _pack_core(q_r, k_r, v_r, wq_r, wk_r, wv_r, wo_r, bpl):
    """One core's payload: q/k/v rows [1024,1024], W rows [128,1024],
    bias plane [4,2048] uint8 -> [PAY] uint8."""
    qp_, qs = _quant4(q_r, SQ_QKV)
    kp_, ks = _quant4(k_r, SQ_QKV)
    vp_, vs = _quant4(v_r, SQ_QKV)
    wn, wsc = [], []
    for w in (wq_r, wk_r, wv_r, wo_r):
        n, sc = _quant4(w, SQ_W)
        wn.append(n.reshape(-1))
        wsc.append(sc.reshape(-1))
    return jnp.concatenate([
        qp_.reshape(-1), kp_.reshape(-1), vp_.reshape(-1),
        qs.reshape(-1), ks.reshape(-1), vs.reshape(-1),
        *wn, *wsc, bpl.reshape(-1),
    ])


# ---------------- host-side unpack (per-shard, numpy) ----------------

def _unpack_shard(buf, bo, out, c):
    """buf [1024,544] uint8 -> f32 rows written into out[batch, soff:]."""
    p = buf[:, :D // 2]
    n = np.empty((ROWS, D), np.float32)
    n[:, :D // 2] = (p & 15).astype(np.float32)
    n[:, D // 2:] = (p >> 4).astype(np.float32)
    n -= 8.0
    slo = buf[:, D // 2:D // 2 + 16].astype(np.uint16)
    shi = buf[:, D // 2 + 16:].astype(np.uint16)
    s = ((slo | (shi << 8)).astype(np.float32)) * SQ_OUT     # [1024,16]
    y = n.reshape(ROWS, H, Hd)
    y *= s[:, :, None]
    res = y.reshape(ROWS, D)
    res += bo[None, :]
    out[c // 2, (c % 2) * ROWS:(c % 2) * ROWS + ROWS] = res


# ---------------- device-side decode/compute/encode ----------------

def _dec_scales(plane, quant, rows):
    pl = plane.reshape(rows, 32).astype(jnp.float32)
    return (pl[:, :16] + pl[:, 16:] * 256.0) * quant


def _dec_nib4(pb, splane, quant, rows):
    """packed nibbles [rows*512] + scale plane -> [rows,1024] f32."""
    p = pb.reshape(rows, D // 2).astype(jnp.float32)
    hi = jnp.floor(p * (1.0 / 16.0))
    lo = p - hi * 16.0
    n = jnp.concatenate([lo, hi], axis=1) - 8.0
    s = _dec_scales(splane, quant, rows)
    return (n.reshape(rows, H, Hd) * s[:, :, None]).reshape(rows, D)


def _core(pay):
    pay = pay.reshape(PAY)

    qf = _dec_nib4(pay[OFF_Q:OFF_Q + _QNIB], pay[OFF_QS:OFF_QS + _SCL],
                   SQ_QKV, ROWS)
    kf = _dec_nib4(pay[OFF_K:OFF_K + _QNIB], pay[OFF_KS:OFF_KS + _SCL],
                   SQ_QKV, ROWS)
    vf = _dec_nib4(pay[OFF_V:OFF_V + _QNIB], pay[OFF_VS:OFF_VS + _SCL],
                   SQ_QKV, ROWS)

    Ws = []
    for t in range(4):
        w_sh = _dec_nib4(pay[OFF_W + t * _WNIB:OFF_W + (t + 1) * _WNIB],
                         pay[OFF_WS + t * _WSCL:OFF_WS + (t + 1) * _WSCL],
                         SQ_W, WROWS)
        Ws.append(jax.lax.all_gather(w_sh, 'x', tiled=True))  # [1024,1024]
    Wq, Wk, Wv, Wo = Ws

    bpl = pay[OFF_B:OFF_B + _BPL].reshape(4, 2 * D).astype(jnp.float32)
    bia = (bpl[:, :D] + bpl[:, D:] * 256.0) * SQ_B - (32768.0 * SQ_B)
    bq, bk, bv = bia[0], bia[1], bia[2]          # bia[3]=bo added on host

    qp = (qf @ Wq.T + bq).reshape(ROWS, H, Hd)
    kp = (kf @ Wk.T + bk).reshape(ROWS, H, Hd)
    vp = (vf @ Wv.T + bv).reshape(ROWS, H, Hd)

    # one-hot circulant helpers, built on device
    i3 = jax.lax.broadcasted_iota(jnp.int32, (Hd, Hd, Hd), 0)
    m3 = jax.lax.broadcasted_iota(jnp.int32, (Hd, Hd, Hd), 1)
    j3 = jax.lax.broadcasted_iota(jnp.int32, (Hd, Hd, Hd), 2)
    M = ((i3 + m3 - j3) % Hd == 0).astype(jnp.float32)
    i2 = jax.lax.broadcasted_iota(jnp.int32, (Hd, Hd), 0)
    j2 = jax.lax.broadcasted_iota(jnp.int32, (Hd, Hd), 1)
    Pm = ((i2 + j2) % Hd == 0).astype(jnp.float32)

    # bind: G[h,i,m] = sum_local_s kp[s,h,i] vp[s,h,m]; psum over the pair
    G = jnp.einsum('shi,shm->him', kp, vp)
    G = jax.lax.psum(G, 'x', axis_index_groups=PAIRS)
    beta = G.reshape(H, Hd * Hd) @ M.reshape(Hd * Hd, Hd)    # [H,Hd]

    # unbind: qt = qp @ P ; Cbeta[h,m,j] = beta[h,(j-m)%64]
    qt = jnp.einsum('shm,mj->shj', qp, Pm)
    Cbeta = (beta @ M.reshape(Hd, Hd * Hd)).reshape(H, Hd, Hd)
    v_hat = jnp.einsum('shm,hmj->shj', qt, Cbeta)            # [ROWS,H,Hd]

    # cosine similarity along Hd (clamp each norm at eps)
    dot = (vp * v_hat).sum(-1)
    nv = jnp.maximum(jnp.sqrt((vp * vp).sum(-1)), EPS)
    nh = jnp.maximum(jnp.sqrt((v_hat * v_hat).sum(-1)), EPS)
    a = dot / (nv * nh)                                      # [ROWS,H]

    # softmax over S = the two cores of this pair
    m_loc = a.max(axis=0)
    m_glob = jax.lax.pmax(m_loc, 'x', axis_index_groups=PAIRS)
    e = jnp.exp(a - m_glob)
    s_loc = e.sum(axis=0)
    s_glob = jax.lax.psum(s_loc, 'x', axis_index_groups=PAIRS)
    w = e / s_glob                                           # [ROWS,H]

    attn = (w[..., None] * vp).reshape(ROWS, D)
    y = attn @ Wo.T                                          # NO bo here

    # int4 encode with per-64-block scales, uint16 fixed-point planes
    yb = y.reshape(ROWS, H, Hd)
    am = jnp.max(jnp.abs(yb), axis=2)
    s = jnp.clip(am / 7.0, SQ_OUT, 65535.0 * SQ_OUT)
    n = jnp.clip(jnp.round(yb / s[:, :, None]), -7.0, 7.0) + 8.0
    n = n.reshape(ROWS, D)
    pnib = (n[:, :D // 2] + 16.0 * n[:, D // 2:]).astype(jnp.uint8)
    senc = jnp.round(s * (1.0 / SQ_OUT))
    shi = jnp.floor(senc * (1.0 / 256.0))
    slo = senc - shi * 256.0
    return jnp.concatenate([pnib, slo.astype(jnp.uint8),
                            shi.astype(jnp.uint8)], axis=1)  # [ROWS,544]


@jax.jit
def _spmd(pay):
    mesh, _ = _init_mesh()
    f = shard_map(_core, mesh=mesh, in_specs=(P('x', None),),
                  out_specs=P('x', None), **_SM_KW)
    return f(pay)


# ---------------- driver ----------------

def _run_once(q, k, v, Wq, bq, Wk, bk, Wv, bv, Wo, bo):
    mesh, sh_pay = _init_mesh()
    devs = mesh.devices.reshape(-1)

    q = np.asarray(q, np.float32).reshape(B * S, D)
    k = np.asarray(k, np.float32).reshape(B * S, D)
    v = np.asarray(v, np.float32).reshape(B * S, D)
    Wq = np.asarray(Wq, np.float32)
    Wk = np.asarray(Wk, np.float32)
    Wv = np.asarray(Wv, np.float32)
    Wo = np.asarray(Wo, np.float32)
    bo32 = np.asarray(bo, np.float32)

    benc = np.clip(np.round(np.stack([np.asarray(bq, np.float32),
                                      np.asarray(bk, np.float32),
                                      np.asarray(bv, np.float32),
                                      bo32]) / SQ_B) + 32768,
                   0, 65535).astype(np.uint32)
    bpl = np.concatenate([(benc & 255).astype(np.uint8),
                          (benc >> 8).astype(np.uint8)], axis=1)  # [4,2048]

    # pack core c on the CPU while core c-1's shard is on the wire
    shards = []
    for c in range(N):
        r = slice(c * ROWS, (c + 1) * ROWS)
        wr = slice(c * WROWS, (c + 1) * WROWS)
        pay_c = np.asarray(_pack_core(q[r], k[r], v[r], Wq[wr], Wk[wr],
                                      Wv[wr], Wo[wr], bpl))
        shards.append(jax.device_put(pay_c.reshape(1, PAY), devs[c]))

    gpay = jax.make_array_from_single_device_arrays((N, PAY), sh_pay, shards)
    out_pay = _spmd(gpay)

    # fetch shards sequentially (tunnel is serialized); unpack on the CPU
    # while the next shard is in flight
    out = np.empty((B, S, D), np.float32)
    ex = ThreadPoolExecutor(2)
    futs = [ex.submit(lambda sh=sh: np.asarray(sh.data))
            for sh in out_pay.addressable_shards]
    for c, fut in enumerate(futs):
        _unpack_shard(fut.result().reshape(ROWS, OUT_COLS), bo32, out, c)
    ex.shutdown(wait=False)
    return out


def kernel(q, k, v, Wq, bq, Wk, bk, Wv, bv, Wo, bo, **_):
    last = None
    for attempt in range(3):
        try:
            return _run_once(q, k, v, Wq, bq, Wk, bk, Wv, bv, Wo, bo)
        except Exception as e:                      # transient tunnel drops
            last = e
            time.sleep(2.0)
    raise last


# revision 11
# speedup vs baseline: 1.3439x; 1.3439x over previous
"""HRR attention kernel for 8 Trainium2 NeuronCores (axon-tunneled).

The axon host<->device tunnel is the bottleneck (~40 MB/s each way,
serialized across devices, ~110 ms fixed dispatch overhead per SPMD
launch; the host has a single CPU core), so the kernel minimizes wire
bytes and overlaps host work with wire time:

  H2D: ONE uint8 payload [8, PAY] (~15.7 MB), row-sharded, carrying
    - q/k/v int4 (per-64-block scales), two nibbles per byte
    - Wq/Wk/Wv/Wo int4 (per-64-block scales), 128 rows per core
    - scales and biases as uint16 fixed-point lo/hi uint8 planes
  Packing runs per-core on the CPU and each core's shard is put
  asynchronously as soon as it is ready, so pack time hides under the
  serialized wire time of earlier shards.
  D2H: ONE uint8 array [8192, 544]: int4 nibbles of attn @ Wo.T
  WITHOUT bo (the output is ~99% bo; bo is added host-side in f32, so
  the quantization scale only spans the small attention part) plus
  uint16 per-64-block scale planes. Shards are fetched sequentially
  while already-fetched shards unpack on the CPU.

Quantization error budget (vs the CPU reference): int4 qkv+W ~1.9e-3,
int4 output sans bo ~1.1e-3, device compute ~2e-4; total ~2e-3 against
the 2e-2 gate.

Sharding: rows of the flattened [B*S=8192, D] tensors, 1024 rows/core;
core 2b holds batch b s<1024, core 2b+1 batch b s>=1024. Cross-core
reductions (bind-stage sum over S, softmax over S) are psums over core
pairs [[0,1],[2,3],[4,5],[6,7]]. Weight shards all-gather on fabric.

FFT bind/unbind are reformulated as tiny matmuls with one-hot circulant
tensors built on-device from iotas:
  circconv(x, y)[j] = sum_i x[i] y[(j-i)%64]
  bind:   beta[h,j] = sum_{i,m:(i+m)%64==j} G[h,i,m],  G = kp^T @ vp
  unbind: v_hat = qt @ C(beta), C(beta)[m,j] = beta[(j-m)%64]
  approx_transpose: qt = qp @ P, P[i,j] = 1 iff (i+j)%64 == 0.
"""

import os
import time
import ctypes
import hashlib
import subprocess
import numpy as np
import jax
import jax.numpy as jnp
from jax.sharding import Mesh, NamedSharding, PartitionSpec as P
from functools import partial
from concurrent.futures import ThreadPoolExecutor

try:
    from jax import shard_map
    _SM_KW = {'check_vma': False}
except ImportError:
    from jax.experimental.shard_map import shard_map
    _SM_KW = {'check_rep': False}

try:
    jax.config.update("jax_compilation_cache_dir", "/tmp/jax_comp_cache")
    jax.config.update("jax_persistent_cache_min_compile_time_secs", 10.0)
except Exception:
    pass

B, S, D = 4, 2048, 1024
H, Hd = 16, 64
EPS = 1e-8
N = 8
ROWS = B * S // N              # 1024 rows per core
WROWS = D // N                 # 128 weight rows per core
PAIRS = [[0, 1], [2, 3], [4, 5], [6, 7]]

# fixed-point quanta for uint16-encoded scales/biases (clamped on encode)
SQ_QKV = 1e-5                  # qkv block scales ~0.38, max 0.655
SQ_W = 1e-6                    # W int4 block scales ~0.0076, max 0.0655
SQ_B = 4e-6                    # biases ~N(0,0.02^2), offset-binary
SQ_OUT = 1e-6                  # output block scales << 0.0655

# per-core payload layout (offsets in bytes)
_QNIB = ROWS * (D // 2)        # 524288 per qkv tensor
_SCL = ROWS * 32               # 32768: scale lo/hi planes
_WNIB = WROWS * (D // 2)       # 65536 per weight (int4)
_WSCL = WROWS * 32             # 4096 per weight
_BPL = 4 * 2 * D               # 8192: 4 biases, lo+hi planes
OFF_Q, OFF_K, OFF_V = 0, _QNIB, 2 * _QNIB
OFF_QS = 3 * _QNIB
OFF_KS = OFF_QS + _SCL
OFF_VS = OFF_KS + _SCL
OFF_W = OFF_VS + _SCL          # 4 weights contiguous
OFF_WS = OFF_W + 4 * _WNIB
OFF_B = OFF_WS + 4 * _WSCL
PAY = OFF_B + _BPL             # 1957888 (~1.87 MB/core)

OUT_COLS = D // 2 + 32         # 544


_mesh = None
_sh_pay = None
_cpu = None


def _init_mesh():
    global _mesh, _sh_pay
    if _mesh is None:
        devs = jax.devices()[:N]
        _mesh = Mesh(np.array(devs), ('x',))
        _sh_pay = NamedSharding(_mesh, P('x', None))
    return _mesh, _sh_pay


def _get_cpu():
    global _cpu
    if _cpu is None:
        _cpu = jax.devices('cpu')[0]
    return _cpu


# ---------------- C fast path for host pack/unpack ----------------

_C_SRC = r"""
#include <stdint.h>
#include <math.h>

/* quantize [rows,1024] f32 -> int4 nibbles [rows,512] + u16 scale planes
   [rows,32]; block = 64 cols, halves packing: byte j = n[j] | n[512+j]<<4 */
void pack_block(const float* x, long rows, float sq,
                uint8_t* nib, uint8_t* scl) {
    for (long r = 0; r < rows; r++) {
        const float* xr = x + r * 1024;
        uint8_t n[1024];
        for (int h = 0; h < 16; h++) {
            const float* xb = xr + h * 64;
            float am = 0.f;
            for (int j = 0; j < 64; j++) {
                float a = fabsf(xb[j]);
                if (a > am) am = a;
            }
            long enc = (long)ceilf(am / (7.0f * sq));
            if (enc < 1) enc = 1;
            if (enc > 65535) enc = 65535;
            float inv = 1.0f / ((float)enc * sq);
            uint8_t* nb = n + h * 64;
            for (int j = 0; j < 64; j++) {
                int q = (int)(xb[j] * inv + 8.5f);
                if (q < 1) q = 1;
                if (q > 15) q = 15;
                nb[j] = (uint8_t)q;
            }
            scl[r * 32 + h] = (uint8_t)(enc & 255);
            scl[r * 32 + 16 + h] = (uint8_t)(enc >> 8);
        }
        uint8_t* o = nib + r * 512;
        for (int j = 0; j < 512; j++)
            o[j] = (uint8_t)(n[j] | (n[512 + j] << 4));
    }
}

/* buf [rows,544] uint8 -> out [rows,1024] f32 (+= bo), sq = SQ_OUT */
void unpack_out(const uint8_t* buf, const float* bo, float* out,
                long rows, float sq) {
    for (long r = 0; r < rows; r++) {
        const uint8_t* b = buf + r * 544;
        float s[16];
        for (int h = 0; h < 16; h++)
            s[h] = (float)(b[512 + h] | (b[528 + h] << 8)) * sq;
        float* o = out + r * 1024;
        for (int j = 0; j < 512; j++) {
            int lo = (b[j] & 15) - 8;
            int hi = (b[j] >> 4) - 8;
            o[j] = (float)lo * s[j >> 6] + bo[j];
            o[512 + j] = (float)hi * s[(512 + j) >> 6] + bo[512 + j];
        }
    }
}
"""


def _build_clib():
    try:
        h = hashlib.sha1(_C_SRC.encode()).hexdigest()[:16]
        so = f"/tmp/hrr_pack_{h}.so"
        if not os.path.exists(so):
            src = f"/tmp/hrr_pack_{h}.c"
            with open(src, "w") as f:
                f.write(_C_SRC)
            subprocess.run(
                ["cc", "-O3", "-march=native", "-shared", "-fPIC",
                 src, "-o", so, "-lm"],
                check=True, capture_output=True)
        lib = ctypes.CDLL(so)
        u8p = np.ctypeslib.ndpointer(np.uint8, flags="C_CONTIGUOUS")
        f32p = np.ctypeslib.ndpointer(np.float32, flags="C_CONTIGUOUS")
        lib.pack_block.argtypes = [f32p, ctypes.c_long, ctypes.c_float,
                                   u8p, u8p]
        lib.pack_block.restype = None
        lib.unpack_out.argtypes = [u8p, f32p, f32p, ctypes.c_long,
                                   ctypes.c_float]
        lib.unpack_out.restype = None
        # smoke test
        x = np.random.randn(2, 1024).astype(np.float32)
        nib = np.zeros(2 * 512, np.uint8)
        scl = np.zeros(2 * 32, np.uint8)
        lib.pack_block(x, 2, np.float32(SQ_QKV), nib, scl)
        if nib.max() == 0:
            return None
        return lib
    except Exception:
        return None


_clib = _build_clib()


# ---------------- host-side pack (per-core, jit on CPU) ----------------

def _nib_pack(n):
    """n [R,1024] uint8 in [1,15] -> [R,512] packed (byte j = n[j] | n[512+j]<<4)."""
    return n[:, :D // 2] | (n[:, D // 2:] << 4)


def _enc_u16(v, quant):
    """v [R,16] f32 -> [R,32] uint8 lo|hi planes of round(v/quant)."""
    e = jnp.clip(jnp.round(v / quant), 0, 65535).astype(jnp.uint32)
    return jnp.concatenate([(e & 255).astype(jnp.uint8),
                            (e >> 8).astype(jnp.uint8)], axis=1)


def _quant4(x, quant):
    """x [R,1024] f32 -> packed nibbles [R,512], scale planes [R,32]."""
    xb = x.reshape(-1, H, Hd)
    am = jnp.max(jnp.abs(xb), axis=2)
    s = jnp.maximum(am / 7.0, quant)
    n = jnp.clip(jnp.round(xb / s[:, :, None]), -7, 7) + 8
    return _nib_pack(n.reshape(-1, D).astype(jnp.uint8)), _enc_u16(s, quant)


@partial(jax.jit, backend='cpu')
def _pack_core(q_r, k_r, v_r, wq_r, wk_r, wv_r, wo_r, bpl):
    """One core's payload: q/k/v rows [1024,1024], W rows [128,1024],
    bias plane [4,2048] uint8 -> [PAY] uint8."""
    qp_, qs = _quant4(q_r, SQ_QKV)
    kp_, ks = _quant4(k_r, SQ_QKV)
    vp_, vs = _quant4(v_r, SQ_QKV)
    wn, wsc = [], []
    for w in (wq_r, wk_r, wv_r, wo_r):
        n, sc = _quant4(w, SQ_W)
        wn.append(n.reshape(-1))
        wsc.append(sc.reshape(-1))
    return jnp.concatenate([
        qp_.reshape(-1), kp_.reshape(-1), vp_.reshape(-1),
        qs.reshape(-1), ks.reshape(-1), vs.reshape(-1),
        *wn, *wsc, bpl.reshape(-1),
    ])


# ---------------- host-side unpack (per-shard, numpy) ----------------

def _unpack_shard(buf, bo, out, c):
    """buf [1024,544] uint8 -> f32 rows written into out[batch, soff:]."""
    dst = out[c // 2, (c % 2) * ROWS:(c % 2) * ROWS + ROWS]
    if _clib is not None:
        buf = np.ascontiguousarray(buf)
        _clib.unpack_out(buf, bo, dst, ROWS, np.float32(SQ_OUT))
        return
    p = buf[:, :D // 2]
    n = np.empty((ROWS, D), np.float32)
    n[:, :D // 2] = (p & 15).astype(np.float32)
    n[:, D // 2:] = (p >> 4).astype(np.float32)
    n -= 8.0
    slo = buf[:, D // 2:D // 2 + 16].astype(np.uint16)
    shi = buf[:, D // 2 + 16:].astype(np.uint16)
    s = ((slo | (shi << 8)).astype(np.float32)) * SQ_OUT     # [1024,16]
    y = n.reshape(ROWS, H, Hd)
    y *= s[:, :, None]
    res = y.reshape(ROWS, D)
    res += bo[None, :]
    dst[:] = res


# ---------------- device-side decode/compute/encode ----------------

def _dec_scales(plane, quant, rows):
    pl = plane.reshape(rows, 32).astype(jnp.float32)
    return (pl[:, :16] + pl[:, 16:] * 256.0) * quant


def _dec_nib4(pb, splane, quant, rows):
    """packed nibbles [rows*512] + scale plane -> [rows,1024] f32."""
    p = pb.reshape(rows, D // 2).astype(jnp.float32)
    hi = jnp.floor(p * (1.0 / 16.0))
    lo = p - hi * 16.0
    n = jnp.concatenate([lo, hi], axis=1) - 8.0
    s = _dec_scales(splane, quant, rows)
    return (n.reshape(rows, H, Hd) * s[:, :, None]).reshape(rows, D)


def _core(pay):
    pay = pay.reshape(PAY)

    qf = _dec_nib4(pay[OFF_Q:OFF_Q + _QNIB], pay[OFF_QS:OFF_QS + _SCL],
                   SQ_QKV, ROWS)
    kf = _dec_nib4(pay[OFF_K:OFF_K + _QNIB], pay[OFF_KS:OFF_KS + _SCL],
                   SQ_QKV, ROWS)
    vf = _dec_nib4(pay[OFF_V:OFF_V + _QNIB], pay[OFF_VS:OFF_VS + _SCL],
                   SQ_QKV, ROWS)

    Ws = []
    for t in range(4):
        w_sh = _dec_nib4(pay[OFF_W + t * _WNIB:OFF_W + (t + 1) * _WNIB],
                         pay[OFF_WS + t * _WSCL:OFF_WS + (t + 1) * _WSCL],
                         SQ_W, WROWS)
        Ws.append(jax.lax.all_gather(w_sh, 'x', tiled=True))  # [1024,1024]
    Wq, Wk, Wv, Wo = Ws

    bpl = pay[OFF_B:OFF_B + _BPL].reshape(4, 2 * D).astype(jnp.float32)
    bia = (bpl[:, :D] + bpl[:, D:] * 256.0) * SQ_B - (32768.0 * SQ_B)
    bq, bk, bv = bia[0], bia[1], bia[2]          # bia[3]=bo added on host

    qp = (qf @ Wq.T + bq).reshape(ROWS, H, Hd)
    kp = (kf @ Wk.T + bk).reshape(ROWS, H, Hd)
    vp = (vf @ Wv.T + bv).reshape(ROWS, H, Hd)

    # one-hot circulant helpers, built on device
    i3 = jax.lax.broadcasted_iota(jnp.int32, (Hd, Hd, Hd), 0)
    m3 = jax.lax.broadcasted_iota(jnp.int32, (Hd, Hd, Hd), 1)
    j3 = jax.lax.broadcasted_iota(jnp.int32, (Hd, Hd, Hd), 2)
    M = ((i3 + m3 - j3) % Hd == 0).astype(jnp.float32)
    i2 = jax.lax.broadcasted_iota(jnp.int32, (Hd, Hd), 0)
    j2 = jax.lax.broadcasted_iota(jnp.int32, (Hd, Hd), 1)
    Pm = ((i2 + j2) % Hd == 0).astype(jnp.float32)

    # bind: G[h,i,m] = sum_local_s kp[s,h,i] vp[s,h,m]; psum over the pair
    G = jnp.einsum('shi,shm->him', kp, vp)
    G = jax.lax.psum(G, 'x', axis_index_groups=PAIRS)
    beta = G.reshape(H, Hd * Hd) @ M.reshape(Hd * Hd, Hd)    # [H,Hd]

    # unbind: qt = qp @ P ; Cbeta[h,m,j] = beta[h,(j-m)%64]
    qt = jnp.einsum('shm,mj->shj', qp, Pm)
    Cbeta = (beta @ M.reshape(Hd, Hd * Hd)).reshape(H, Hd, Hd)
    v_hat = jnp.einsum('shm,hmj->shj', qt, Cbeta)            # [ROWS,H,Hd]

    # cosine similarity along Hd (clamp each norm at eps)
    dot = (vp * v_hat).sum(-1)
    nv = jnp.maximum(jnp.sqrt((vp * vp).sum(-1)), EPS)
    nh = jnp.maximum(jnp.sqrt((v_hat * v_hat).sum(-1)), EPS)
    a = dot / (nv * nh)                                      # [ROWS,H]

    # softmax over S = the two cores of this pair
    m_loc = a.max(axis=0)
    m_glob = jax.lax.pmax(m_loc, 'x', axis_index_groups=PAIRS)
    e = jnp.exp(a - m_glob)
    s_loc = e.sum(axis=0)
    s_glob = jax.lax.psum(s_loc, 'x', axis_index_groups=PAIRS)
    w = e / s_glob                                           # [ROWS,H]

    attn = (w[..., None] * vp).reshape(ROWS, D)
    y = attn @ Wo.T                                          # NO bo here

    # int4 encode with per-64-block scales, uint16 fixed-point planes
    yb = y.reshape(ROWS, H, Hd)
    am = jnp.max(jnp.abs(yb), axis=2)
    s = jnp.clip(am / 7.0, SQ_OUT, 65535.0 * SQ_OUT)
    n = jnp.clip(jnp.round(yb / s[:, :, None]), -7.0, 7.0) + 8.0
    n = n.reshape(ROWS, D)
    pnib = (n[:, :D // 2] + 16.0 * n[:, D // 2:]).astype(jnp.uint8)
    senc = jnp.round(s * (1.0 / SQ_OUT))
    shi = jnp.floor(senc * (1.0 / 256.0))
    slo = senc - shi * 256.0
    return jnp.concatenate([pnib, slo.astype(jnp.uint8),
                            shi.astype(jnp.uint8)], axis=1)  # [ROWS,544]


@jax.jit
def _spmd(pay):
    mesh, _ = _init_mesh()
    f = shard_map(_core, mesh=mesh, in_specs=(P('x', None),),
                  out_specs=P('x', None), **_SM_KW)
    return f(pay)


# ---------------- driver ----------------

def _run_once(q, k, v, Wq, bq, Wk, bk, Wv, bv, Wo, bo):
    mesh, sh_pay = _init_mesh()
    devs = mesh.devices.reshape(-1)

    q = np.asarray(q, np.float32).reshape(B * S, D)
    k = np.asarray(k, np.float32).reshape(B * S, D)
    v = np.asarray(v, np.float32).reshape(B * S, D)
    Wq = np.asarray(Wq, np.float32)
    Wk = np.asarray(Wk, np.float32)
    Wv = np.asarray(Wv, np.float32)
    Wo = np.asarray(Wo, np.float32)
    bo32 = np.asarray(bo, np.float32)

    benc = np.clip(np.round(np.stack([np.asarray(bq, np.float32),
                                      np.asarray(bk, np.float32),
                                      np.asarray(bv, np.float32),
                                      bo32]) / SQ_B) + 32768,
                   0, 65535).astype(np.uint32)
    bpl = np.concatenate([(benc & 255).astype(np.uint8),
                          (benc >> 8).astype(np.uint8)], axis=1)  # [4,2048]

    # pack core c on the CPU while core c-1's shard is on the wire
    shards = []
    for c in range(N):
        r = slice(c * ROWS, (c + 1) * ROWS)
        wr = slice(c * WROWS, (c + 1) * WROWS)
        if _clib is not None:
            pay_c = np.empty(PAY, np.uint8)
            pb = _clib.pack_block
            pb(q[r], ROWS, np.float32(SQ_QKV),
               pay_c[OFF_Q:OFF_Q + _QNIB], pay_c[OFF_QS:OFF_QS + _SCL])
            pb(k[r], ROWS, np.float32(SQ_QKV),
               pay_c[OFF_K:OFF_K + _QNIB], pay_c[OFF_KS:OFF_KS + _SCL])
            pb(v[r], ROWS, np.float32(SQ_QKV),
               pay_c[OFF_V:OFF_V + _QNIB], pay_c[OFF_VS:OFF_VS + _SCL])
            for t, W in enumerate((Wq, Wk, Wv, Wo)):
                pb(W[wr], WROWS, np.float32(SQ_W),
                   pay_c[OFF_W + t * _WNIB:OFF_W + (t + 1) * _WNIB],
                   pay_c[OFF_WS + t * _WSCL:OFF_WS + (t + 1) * _WSCL])
            pay_c[OFF_B:] = bpl.reshape(-1)
        else:
            pay_c = np.asarray(_pack_core(q[r], k[r], v[r], Wq[wr], Wk[wr],
                                          Wv[wr], Wo[wr], bpl))
        shards.append(jax.device_put(pay_c.reshape(1, PAY), devs[c]))

    gpay = jax.make_array_from_single_device_arrays((N, PAY), sh_pay, shards)
    out_pay = _spmd(gpay)

    # fetch shards sequentially (tunnel is serialized); unpack on the CPU
    # while the next shard is in flight
    out = np.empty((B, S, D), np.float32)
    ex = ThreadPoolExecutor(2)
    futs = [ex.submit(lambda sh=sh: np.asarray(sh.data))
            for sh in out_pay.addressable_shards]
    for c, fut in enumerate(futs):
        _unpack_shard(fut.result().reshape(ROWS, OUT_COLS), bo32, out, c)
    ex.shutdown(wait=False)
    return out


def kernel(q, k, v, Wq, bq, Wk, bk, Wv, bv, Wo, bo, **_):
    last = None
    for attempt in range(3):
        try:
            return _run_once(q, k, v, Wq, bq, Wk, bk, Wv, bv, Wo, bo)
        except Exception as e:                      # transient tunnel drops
            last = e
            time.sleep(2.0)
    raise last


# revision 13
# speedup vs baseline: 1.7451x; 1.2985x over previous
"""HRR attention kernel for 8 Trainium2 NeuronCores (axon-tunneled).

The axon host<->device tunnel is the bottleneck (~40 MB/s each way,
serialized across devices, ~110 ms fixed dispatch overhead per SPMD
launch; the host has a single CPU core), so the kernel minimizes wire
bytes and overlaps host work with wire time:

  H2D: ONE uint8 payload [8, PAY] (~15.7 MB), row-sharded, carrying
    - q/k/v int4 (per-64-block scales), two nibbles per byte
    - Wq/Wk/Wv/Wo int4 (per-64-block scales), 128 rows per core
    - scales and biases as uint16 fixed-point lo/hi uint8 planes
  Packing runs per-core on the CPU and each core's shard is put
  asynchronously as soon as it is ready, so pack time hides under the
  serialized wire time of earlier shards.
  D2H: ONE uint8 array [8192, 544]: int4 nibbles of attn @ Wo.T
  WITHOUT bo (the output is ~99% bo; bo is added host-side in f32, so
  the quantization scale only spans the small attention part) plus
  uint16 per-64-block scale planes. Shards are fetched sequentially
  while already-fetched shards unpack on the CPU.

Quantization error budget (vs the CPU reference): int4 qkv+W ~1.9e-3,
int4 output sans bo ~1.1e-3, device compute ~2e-4; total ~2e-3 against
the 2e-2 gate.

Sharding: rows of the flattened [B*S=8192, D] tensors, 1024 rows/core;
core 2b holds batch b s<1024, core 2b+1 batch b s>=1024. Cross-core
reductions (bind-stage sum over S, softmax over S) are psums over core
pairs [[0,1],[2,3],[4,5],[6,7]]. Weight shards all-gather on fabric.

FFT bind/unbind are reformulated as tiny matmuls with one-hot circulant
tensors built on-device from iotas:
  circconv(x, y)[j] = sum_i x[i] y[(j-i)%64]
  bind:   beta[h,j] = sum_{i,m:(i+m)%64==j} G[h,i,m],  G = kp^T @ vp
  unbind: v_hat = qt @ C(beta), C(beta)[m,j] = beta[(j-m)%64]
  approx_transpose: qt = qp @ P, P[i,j] = 1 iff (i+j)%64 == 0.
"""

import os
import time
import ctypes
import hashlib
import subprocess
import numpy as np
import jax
import jax.numpy as jnp
from jax.sharding import Mesh, NamedSharding, PartitionSpec as P
from functools import partial
from concurrent.futures import ThreadPoolExecutor

try:
    from jax import shard_map
    _SM_KW = {'check_vma': False}
except ImportError:
    from jax.experimental.shard_map import shard_map
    _SM_KW = {'check_rep': False}

try:
    jax.config.update("jax_compilation_cache_dir", "/tmp/jax_comp_cache")
    jax.config.update("jax_persistent_cache_min_compile_time_secs", 10.0)
except Exception:
    pass

B, S, D = 4, 2048, 1024
H, Hd = 16, 64
EPS = 1e-8
N = 8
ROWS = B * S // N              # 1024 rows per core
WROWS = D // N                 # 128 weight rows per core
PAIRS = [[0, 1], [2, 3], [4, 5], [6, 7]]

# fixed-point quanta for uint16-encoded scales/biases (clamped on encode)
SQ_QKV = 1e-5                  # qkv block scales ~0.38, max 0.655
SQ_W = 1e-6                    # W int4 block scales ~0.0076, max 0.0655
SQ_B = 4e-6                    # biases ~N(0,0.02^2), offset-binary
SQ_OUT = 1e-6                  # output block scales << 0.0655

# per-core payload layout (offsets in bytes)
_QNIB = ROWS * (D // 2)        # 524288 per qkv tensor
_SCL = ROWS * 32               # 32768: scale lo/hi planes
_WNIB = WROWS * (D // 2)       # 65536 per weight (int4)
_WSCL = WROWS * 32             # 4096 per weight
_BPL = 4 * 2 * D               # 8192: 4 biases, lo+hi planes
OFF_Q, OFF_K, OFF_V = 0, _QNIB, 2 * _QNIB
OFF_QS = 3 * _QNIB
OFF_KS = OFF_QS + _SCL
OFF_VS = OFF_KS + _SCL
OFF_W = OFF_VS + _SCL          # 4 weights contiguous
OFF_WS = OFF_W + 4 * _WNIB
OFF_B = OFF_WS + 4 * _WSCL
PAY = OFF_B + _BPL             # 1957888 (~1.87 MB/core)

OUT_COLS = D // 2 + 32         # 544


_mesh = None
_sh_pay = None
_cpu = None


def _init_mesh():
    global _mesh, _sh_pay
    if _mesh is None:
        devs = jax.devices()[:N]
        _mesh = Mesh(np.array(devs), ('x',))
        _sh_pay = NamedSharding(_mesh, P('x', None))
    return _mesh, _sh_pay


def _get_cpu():
    global _cpu
    if _cpu is None:
        _cpu = jax.devices('cpu')[0]
    return _cpu


# ---------------- C fast path for host pack/unpack ----------------

_C_SRC = r"""
#include <stdint.h>
#include <math.h>

/* quantize [rows,1024] f32 -> int4 nibbles [rows,512] + u16 scale planes
   [rows,32]; block = 64 cols, halves packing: byte j = n[j] | n[512+j]<<4 */
void pack_block(const float* x, long rows, float sq,
                uint8_t* nib, uint8_t* scl) {
    for (long r = 0; r < rows; r++) {
        const float* xr = x + r * 1024;
        uint8_t n[1024];
        for (int h = 0; h < 16; h++) {
            const float* xb = xr + h * 64;
            float am = 0.f;
            for (int j = 0; j < 64; j++) {
                float a = fabsf(xb[j]);
                if (a > am) am = a;
            }
            long enc = (long)ceilf(am / (7.0f * sq));
            if (enc < 1) enc = 1;
            if (enc > 65535) enc = 65535;
            float inv = 1.0f / ((float)enc * sq);
            uint8_t* nb = n + h * 64;
            for (int j = 0; j < 64; j++) {
                int q = (int)(xb[j] * inv + 8.5f);
                if (q < 1) q = 1;
                if (q > 15) q = 15;
                nb[j] = (uint8_t)q;
            }
            scl[r * 32 + h] = (uint8_t)(enc & 255);
            scl[r * 32 + 16 + h] = (uint8_t)(enc >> 8);
        }
        uint8_t* o = nib + r * 512;
        for (int j = 0; j < 512; j++)
            o[j] = (uint8_t)(n[j] | (n[512 + j] << 4));
    }
}

/* buf [rows,544] uint8 -> out [rows,1024] f32 (+= bo), sq = SQ_OUT */
void unpack_out(const uint8_t* buf, const float* bo, float* out,
                long rows, float sq) {
    for (long r = 0; r < rows; r++) {
        const uint8_t* b = buf + r * 544;
        float s[16];
        for (int h = 0; h < 16; h++)
            s[h] = (float)(b[512 + h] | (b[528 + h] << 8)) * sq;
        float* o = out + r * 1024;
        for (int j = 0; j < 512; j++) {
            int lo = (b[j] & 15) - 8;
            int hi = (b[j] >> 4) - 8;
            o[j] = (float)lo * s[j >> 6] + bo[j];
            o[512 + j] = (float)hi * s[(512 + j) >> 6] + bo[512 + j];
        }
    }
}
"""


def _build_clib():
    try:
        h = hashlib.sha1(_C_SRC.encode()).hexdigest()[:16]
        so = f"/tmp/hrr_pack_{h}.so"
        if not os.path.exists(so):
            src = f"/tmp/hrr_pack_{h}.c"
            with open(src, "w") as f:
                f.write(_C_SRC)
            subprocess.run(
                ["cc", "-O3", "-march=native", "-shared", "-fPIC",
                 src, "-o", so, "-lm"],
                check=True, capture_output=True)
        lib = ctypes.CDLL(so)
        u8p = np.ctypeslib.ndpointer(np.uint8, flags="C_CONTIGUOUS")
        f32p = np.ctypeslib.ndpointer(np.float32, flags="C_CONTIGUOUS")
        lib.pack_block.argtypes = [f32p, ctypes.c_long, ctypes.c_float,
                                   u8p, u8p]
        lib.pack_block.restype = None
        lib.unpack_out.argtypes = [u8p, f32p, f32p, ctypes.c_long,
                                   ctypes.c_float]
        lib.unpack_out.restype = None
        # smoke test
        x = np.random.randn(2, 1024).astype(np.float32)
        nib = np.zeros(2 * 512, np.uint8)
        scl = np.zeros(2 * 32, np.uint8)
        lib.pack_block(x, 2, np.float32(SQ_QKV), nib, scl)
        if nib.max() == 0:
            return None
        return lib
    except Exception:
        return None


_clib = _build_clib()


# ---------------- host-side pack (per-core, jit on CPU) ----------------

def _nib_pack(n):
    """n [R,1024] uint8 in [1,15] -> [R,512] packed (byte j = n[j] | n[512+j]<<4)."""
    return n[:, :D // 2] | (n[:, D // 2:] << 4)


def _enc_u16(v, quant):
    """v [R,16] f32 -> [R,32] uint8 lo|hi planes of round(v/quant)."""
    e = jnp.clip(jnp.round(v / quant), 0, 65535).astype(jnp.uint32)
    return jnp.concatenate([(e & 255).astype(jnp.uint8),
                            (e >> 8).astype(jnp.uint8)], axis=1)


def _quant4(x, quant):
    """x [R,1024] f32 -> packed nibbles [R,512], scale planes [R,32]."""
    xb = x.reshape(-1, H, Hd)
    am = jnp.max(jnp.abs(xb), axis=2)
    s = jnp.maximum(am / 7.0, quant)
    n = jnp.clip(jnp.round(xb / s[:, :, None]), -7, 7) + 8
    return _nib_pack(n.reshape(-1, D).astype(jnp.uint8)), _enc_u16(s, quant)


@partial(jax.jit, backend='cpu')
def _pack_core(q_r, k_r, v_r, wq_r, wk_r, wv_r, wo_r, bpl):
    """One core's payload: q/k/v rows [1024,1024], W rows [128,1024],
    bias plane [4,2048] uint8 -> [PAY] uint8."""
    qp_, qs = _quant4(q_r, SQ_QKV)
    kp_, ks = _quant4(k_r, SQ_QKV)
    vp_, vs = _quant4(v_r, SQ_QKV)
    wn, wsc = [], []
    for w in (wq_r, wk_r, wv_r, wo_r):
        n, sc = _quant4(w, SQ_W)
        wn.append(n.reshape(-1))
        wsc.append(sc.reshape(-1))
    return jnp.concatenate([
        qp_.reshape(-1), kp_.reshape(-1), vp_.reshape(-1),
        qs.reshape(-1), ks.reshape(-1), vs.reshape(-1),
        *wn, *wsc, bpl.reshape(-1),
    ])


# ---------------- host-side unpack (per-shard, numpy) ----------------

def _unpack_shard(buf, bo, out, c):
    """buf [1024,544] uint8 -> f32 rows written into out[batch, soff:]."""
    dst = out[c // 2, (c % 2) * ROWS:(c % 2) * ROWS + ROWS]
    if _clib is not None:
        buf = np.ascontiguousarray(buf)
        _clib.unpack_out(buf, bo, dst, ROWS, np.float32(SQ_OUT))
        return
    p = buf[:, :D // 2]
    n = np.empty((ROWS, D), np.float32)
    n[:, :D // 2] = (p & 15).astype(np.float32)
    n[:, D // 2:] = (p >> 4).astype(np.float32)
    n -= 8.0
    slo = buf[:, D // 2:D // 2 + 16].astype(np.uint16)
    shi = buf[:, D // 2 + 16:].astype(np.uint16)
    s = ((slo | (shi << 8)).astype(np.float32)) * SQ_OUT     # [1024,16]
    y = n.reshape(ROWS, H, Hd)
    y *= s[:, :, None]
    res = y.reshape(ROWS, D)
    res += bo[None, :]
    dst[:] = res


# ---------------- device-side decode/compute/encode ----------------

def _dec_scales(plane, quant, rows):
    pl = plane.reshape(rows, 32).astype(jnp.float32)
    return (pl[:, :16] + pl[:, 16:] * 256.0) * quant


def _dec_nib4(pb, splane, quant, rows):
    """packed nibbles [rows*512] + scale plane -> [rows,1024] f32."""
    p = pb.reshape(rows, D // 2).astype(jnp.float32)
    hi = jnp.floor(p * (1.0 / 16.0))
    lo = p - hi * 16.0
    n = jnp.concatenate([lo, hi], axis=1) - 8.0
    s = _dec_scales(splane, quant, rows)
    return (n.reshape(rows, H, Hd) * s[:, :, None]).reshape(rows, D)


def _core(pay):
    pay = pay.reshape(PAY)

    qf = _dec_nib4(pay[OFF_Q:OFF_Q + _QNIB], pay[OFF_QS:OFF_QS + _SCL],
                   SQ_QKV, ROWS)
    kf = _dec_nib4(pay[OFF_K:OFF_K + _QNIB], pay[OFF_KS:OFF_KS + _SCL],
                   SQ_QKV, ROWS)
    vf = _dec_nib4(pay[OFF_V:OFF_V + _QNIB], pay[OFF_VS:OFF_VS + _SCL],
                   SQ_QKV, ROWS)

    Ws = []
    for t in range(4):
        w_sh = _dec_nib4(pay[OFF_W + t * _WNIB:OFF_W + (t + 1) * _WNIB],
                         pay[OFF_WS + t * _WSCL:OFF_WS + (t + 1) * _WSCL],
                         SQ_W, WROWS)
        Ws.append(jax.lax.all_gather(w_sh, 'x', tiled=True))  # [1024,1024]
    Wq, Wk, Wv, Wo = Ws

    bpl = pay[OFF_B:OFF_B + _BPL].reshape(4, 2 * D).astype(jnp.float32)
    bia = (bpl[:, :D] + bpl[:, D:] * 256.0) * SQ_B - (32768.0 * SQ_B)
    bq, bk, bv = bia[0], bia[1], bia[2]          # bia[3]=bo added on host

    qp = (qf @ Wq.T + bq).reshape(ROWS, H, Hd)
    kp = (kf @ Wk.T + bk).reshape(ROWS, H, Hd)
    vp = (vf @ Wv.T + bv).reshape(ROWS, H, Hd)

    # one-hot circulant helpers, built on device
    i3 = jax.lax.broadcasted_iota(jnp.int32, (Hd, Hd, Hd), 0)
    m3 = jax.lax.broadcasted_iota(jnp.int32, (Hd, Hd, Hd), 1)
    j3 = jax.lax.broadcasted_iota(jnp.int32, (Hd, Hd, Hd), 2)
    M = ((i3 + m3 - j3) % Hd == 0).astype(jnp.float32)
    i2 = jax.lax.broadcasted_iota(jnp.int32, (Hd, Hd), 0)
    j2 = jax.lax.broadcasted_iota(jnp.int32, (Hd, Hd), 1)
    Pm = ((i2 + j2) % Hd == 0).astype(jnp.float32)

    # bind: G[h,i,m] = sum_local_s kp[s,h,i] vp[s,h,m]; psum over the pair
    G = jnp.einsum('shi,shm->him', kp, vp)
    G = jax.lax.psum(G, 'x', axis_index_groups=PAIRS)
    beta = G.reshape(H, Hd * Hd) @ M.reshape(Hd * Hd, Hd)    # [H,Hd]

    # unbind: qt = qp @ P ; Cbeta[h,m,j] = beta[h,(j-m)%64]
    qt = jnp.einsum('shm,mj->shj', qp, Pm)
    Cbeta = (beta @ M.reshape(Hd, Hd * Hd)).reshape(H, Hd, Hd)
    v_hat = jnp.einsum('shm,hmj->shj', qt, Cbeta)            # [ROWS,H,Hd]

    # cosine similarity along Hd (clamp each norm at eps)
    dot = (vp * v_hat).sum(-1)
    nv = jnp.maximum(jnp.sqrt((vp * vp).sum(-1)), EPS)
    nh = jnp.maximum(jnp.sqrt((v_hat * v_hat).sum(-1)), EPS)
    a = dot / (nv * nh)                                      # [ROWS,H]

    # softmax over S = the two cores of this pair
    m_loc = a.max(axis=0)
    m_glob = jax.lax.pmax(m_loc, 'x', axis_index_groups=PAIRS)
    e = jnp.exp(a - m_glob)
    s_loc = e.sum(axis=0)
    s_glob = jax.lax.psum(s_loc, 'x', axis_index_groups=PAIRS)
    w = e / s_glob                                           # [ROWS,H]

    attn = (w[..., None] * vp).reshape(ROWS, D)
    y = attn @ Wo.T                                          # NO bo here

    # int4 encode with per-64-block scales, uint16 fixed-point planes
    yb = y.reshape(ROWS, H, Hd)
    am = jnp.max(jnp.abs(yb), axis=2)
    s = jnp.clip(am / 7.0, SQ_OUT, 65535.0 * SQ_OUT)
    n = jnp.clip(jnp.round(yb / s[:, :, None]), -7.0, 7.0) + 8.0
    n = n.reshape(ROWS, D)
    pnib = (n[:, :D // 2] + 16.0 * n[:, D // 2:]).astype(jnp.uint8)
    senc = jnp.round(s * (1.0 / SQ_OUT))
    shi = jnp.floor(senc * (1.0 / 256.0))
    slo = senc - shi * 256.0
    return jnp.concatenate([pnib, slo.astype(jnp.uint8),
                            shi.astype(jnp.uint8)], axis=1)  # [ROWS,544]


def _core_ag(pay):
    # all-gather the per-core outputs on fabric so the host fetches the
    # whole result from ONE device (one RPC instead of 8)
    return jax.lax.all_gather(_core(pay), 'x', tiled=True)   # [B*S,544]


@jax.jit
def _spmd(pay):
    mesh, _ = _init_mesh()
    f = shard_map(_core_ag, mesh=mesh, in_specs=(P('x', None),),
                  out_specs=P(None, None), **_SM_KW)
    return f(pay)


# ---------------- driver ----------------

def _run_once(q, k, v, Wq, bq, Wk, bk, Wv, bv, Wo, bo):
    mesh, sh_pay = _init_mesh()
    devs = mesh.devices.reshape(-1)

    q = np.asarray(q, np.float32).reshape(B * S, D)
    k = np.asarray(k, np.float32).reshape(B * S, D)
    v = np.asarray(v, np.float32).reshape(B * S, D)
    Wq = np.asarray(Wq, np.float32)
    Wk = np.asarray(Wk, np.float32)
    Wv = np.asarray(Wv, np.float32)
    Wo = np.asarray(Wo, np.float32)
    bo32 = np.asarray(bo, np.float32)

    benc = np.clip(np.round(np.stack([np.asarray(bq, np.float32),
                                      np.asarray(bk, np.float32),
                                      np.asarray(bv, np.float32),
                                      bo32]) / SQ_B) + 32768,
                   0, 65535).astype(np.uint32)
    bpl = np.concatenate([(benc & 255).astype(np.uint8),
                          (benc >> 8).astype(np.uint8)], axis=1)  # [4,2048]

    # pack core c on the CPU while core c-1's shard is on the wire
    shards = []
    for c in range(N):
        r = slice(c * ROWS, (c + 1) * ROWS)
        wr = slice(c * WROWS, (c + 1) * WROWS)
        if _clib is not None:
            pay_c = np.empty(PAY, np.uint8)
            pb = _clib.pack_block
            pb(q[r], ROWS, np.float32(SQ_QKV),
               pay_c[OFF_Q:OFF_Q + _QNIB], pay_c[OFF_QS:OFF_QS + _SCL])
            pb(k[r], ROWS, np.float32(SQ_QKV),
               pay_c[OFF_K:OFF_K + _QNIB], pay_c[OFF_KS:OFF_KS + _SCL])
            pb(v[r], ROWS, np.float32(SQ_QKV),
               pay_c[OFF_V:OFF_V + _QNIB], pay_c[OFF_VS:OFF_VS + _SCL])
            for t, W in enumerate((Wq, Wk, Wv, Wo)):
                pb(W[wr], WROWS, np.float32(SQ_W),
                   pay_c[OFF_W + t * _WNIB:OFF_W + (t + 1) * _WNIB],
                   pay_c[OFF_WS + t * _WSCL:OFF_WS + (t + 1) * _WSCL])
            pay_c[OFF_B:] = bpl.reshape(-1)
        else:
            pay_c = np.asarray(_pack_core(q[r], k[r], v[r], Wq[wr], Wk[wr],
                                          Wv[wr], Wo[wr], bpl))
        shards.append(jax.device_put(pay_c.reshape(1, PAY), devs[c]))

    gpay = jax.make_array_from_single_device_arrays((N, PAY), sh_pay, shards)
    out_pay = _spmd(gpay)

    buf = np.asarray(out_pay.addressable_shards[0].data)     # one 4.5MB RPC
    out = np.empty((B, S, D), np.float32)
    for c in range(N):
        _unpack_shard(buf[c * ROWS:(c + 1) * ROWS], bo32, out, c)
    return out


def kernel(q, k, v, Wq, bq, Wk, bk, Wv, bv, Wo, bo, **_):
    last = None
    for attempt in range(3):
        try:
            return _run_once(q, k, v, Wq, bq, Wk, bk, Wv, bv, Wo, bo)
        except Exception as e:                      # transient tunnel drops
            last = e
            time.sleep(2.0)
    raise last
